# revision 1
# baseline (speedup 1.0000x reference)
"""Fused sparse-attention kernel for Trainium2 — 8-core SPMD, data-parallel over batch.

Reference computation (per call, two calls: (V, r_i) and (T, r_t)):
    q      = x @ Wq.T + bq                      # [b,256,768]
    k      = r @ Wk.T + bk                      # [b,8,256,768]
    v      = r @ Wv.T + bv
    S      = (q @ k.T) / sqrt(768)              # [b,8,256,256]
    P      = softmax(S, -1)
    out    = mean_k( pool16(P @ v) )            # pool16: avg over groups of 16 q rows

Algebraic restructuring used here:
  1. softmax is shift-invariant => the bk bias term (constant along s) drops.
     S = q' @ r.T with q' = x @ Wqk + bqk,  Wqk = Wq.T @ Wk * scale (host-precomputed).
     This removes the [b,8,256,768]x[768,768] k-projection entirely.
  2. pool16 is linear => apply it to P before the value matmul:
     pool16(P @ v) = (pool16 P) @ v.  16x fewer FLOPs in the AV matmul.
  3. Wv/bv projection commutes with the k-mean and the pool:
     out = ( mean_k (pool16 P) @ r ) @ Wv.T + bv.  One value projection per batch
     instead of per (batch, k); bv survives unscaled because pooled probs rows sum to 1.
  4. pooling + softmax normalization + k-mean are folded into one tiny matmul:
     Ppooled.T[s, p] = sum_q E[q, s] * M[q, p] / (16*8*rowsum[q]), with
     E = exp(S) and M the 0/1 pool-bin scatter matrix (host constant).

Total ~76 GFLOP instead of ~435 GFLOP.

Per-core dataflow (bf16 matmul inputs, fp32 accumulation):
  - x, r loaded with SWDGE cast f32->bf16.
  - r.T tiles for the scores matmul are produced with the DMA xbar transpose
    (SBUF->SBUF, bf16).
  - exp on ScalarE reads scores straight from PSUM and emits the softmax
    denominator via accum_out in the same pass.
"""

import numpy as np
import ml_dtypes

B, K, S, SQ, D = 32, 8, 256, 256, 768
NCORES = 8
BL = B // NCORES          # batches per core
P16 = 16                  # pooled length
NBC = 2 * BL              # (call, batch) units per core
DC = D // 128             # 6 chunks of the feature dim
KH = 4                    # candidates per load/transpose half-group
BF16 = ml_dtypes.bfloat16

_cache = {}


def _build_program():
    import concourse.bass as bass
    import concourse.bacc as bacc
    import concourse.tile as tile
    import concourse.mybir as mybir

    f32 = mybir.dt.float32
    bf16 = mybir.dt.bfloat16
    ts = bass.ts
    AF = mybir.ActivationFunctionType

    nc = bacc.Bacc("TRN2", target_bir_lowering=False, debug=False)

    xv = nc.dram_tensor("xv", [BL, SQ, D], f32, kind="ExternalInput").ap()
    xt = nc.dram_tensor("xt", [BL, SQ, D], f32, kind="ExternalInput").ap()
    rv = nc.dram_tensor("rv", [BL, K, S, D], f32, kind="ExternalInput").ap()
    rt = nc.dram_tensor("rt", [BL, K, S, D], f32, kind="ExternalInput").ap()
    wqk = nc.dram_tensor("wqk", [D, D], bf16, kind="ExternalInput").ap()
    wvt = nc.dram_tensor("wvt", [D, D], bf16, kind="ExternalInput").ap()
    bqk = nc.dram_tensor("bqk", [128, DC], f32, kind="ExternalInput").ap()
    bvc = nc.dram_tensor("bvc", [128, DC], f32, kind="ExternalInput").ap()
    msk = nc.dram_tensor("msk", [SQ, P16], bf16, kind="ExternalInput").ap()
    idn = nc.dram_tensor("idn", [128, 128], bf16, kind="ExternalInput").ap()
    outT = nc.dram_tensor("outT", [D, NBC * P16], f32, kind="ExternalOutput").ap()

    with tile.TileContext(nc) as tc:
        with (
            tc.tile_pool(name="const", bufs=1) as const,
            tc.tile_pool(name="persist", bufs=1) as persist,
            tc.tile_pool(name="bcpool", bufs=2) as bcp,
            tc.tile_pool(name="xpool", bufs=3) as xpool,
            tc.tile_pool(name="rpool", bufs=4) as rpool,
            tc.tile_pool(name="rtpool", bufs=4) as rtpool,
            tc.tile_pool(name="pair", bufs=6) as pair,
            tc.tile_pool(name="ps_scores", bufs=4, space="PSUM") as ps_sc,
            tc.tile_pool(name="ps_u", bufs=1, space="PSUM") as ps_up,
            tc.tile_pool(name="ps_small", bufs=2, space="PSUM") as ps_sm,
        ):
            # ---- constants ----
            wqk_sb = const.tile([128, DC, D], bf16)
            nc.sync.dma_start(wqk_sb[:], wqk.rearrange("(c p) d -> p c d", p=128))
            wvt_sb = const.tile([128, DC, D], bf16)
            nc.sync.dma_start(wvt_sb[:], wvt.rearrange("(c p) d -> p c d", p=128))
            bqk_sb = const.tile([128, DC], f32)
            nc.sync.dma_start(bqk_sb[:], bqk[:])
            bvc_sb = const.tile([128, DC], f32)
            nc.sync.dma_start(bvc_sb[:], bvc[:])
            msk_sb = const.tile([128, 2, P16], bf16)
            nc.sync.dma_start(msk_sb[:], msk.rearrange("(t p) m -> p t m", p=128))
            idn_sb = const.tile([128, 128], bf16)
            nc.sync.dma_start(idn_sb[:], idn[:])

            # transposed, Wv-unprojected pooled outputs for every (call, batch)
            uT_all = persist.tile([128, DC, NBC, P16], bf16)
            U_all = persist.tile([16, NBC, D], bf16)

            def dram_xr(bc):
                call, b = bc // BL, bc % BL
                return (xv, xt)[call][b], (rv, rt)[call][b]

            x_tiles, r_tiles, rT_tiles, qT_tiles = {}, {}, {}, {}

            def issue_x_load(bc):
                x_dram, _ = dram_xr(bc)
                x_sb = xpool.tile([128, 2, D], bf16, tag="xsb")
                nc.gpsimd.dma_start(
                    out=x_sb[:], in_=x_dram.rearrange("(t p) d -> p t d", p=128)
                )
                x_tiles[bc] = x_sb

            def issue_r_loads(bc):
                _, r_dram = dram_xr(bc)
                r_tiles[bc], rT_tiles[bc] = [], []
                for h in range(K // KH):
                    r_re = r_dram[h * KH : (h + 1) * KH].rearrange(
                        "k (t p) d -> p t k d", p=128
                    )
                    r_sb = rpool.tile([128, 2, KH, D], bf16, tag="rsb")
                    rT_sb = rtpool.tile([128, KH, DC, S], bf16, tag="rtsb")
                    for t in range(2):
                        nc.gpsimd.dma_start(out=r_sb[:, t], in_=r_re[:, t])
                    r_tiles[bc].append(r_sb)
                    rT_tiles[bc].append(rT_sb)

            def prologue(bc):
                # x.T via PE transposes (batched eviction per s-chunk),
                # then q'.T = Wqk.T @ x.T (+bias on eviction)
                x_sb = x_tiles.pop(bc)
                xT_sb = bcp.tile([128, DC, SQ], bf16, tag="xT")
                for t in range(2):
                    pst = ps_sm.tile([128, DC, 128], bf16, tag="small")
                    for c in range(DC):
                        nc.tensor.transpose(
                            pst[:, c], x_sb[:, t, ts(c, 128)], idn_sb[:]
                        )
                    nc.vector.tensor_copy(xT_sb[:, :, ts(t, 128)], pst[:])
                qT_sb = bcp.tile([128, DC, SQ], bf16, tag="qT")
                for co in range(DC):
                    psq = ps_sm.tile([128, SQ], f32, tag="small")
                    for ci in range(DC):
                        nc.tensor.matmul(
                            psq[:],
                            lhsT=wqk_sb[:, ci, ts(co, 128)],
                            rhs=xT_sb[:, ci, :],
                            start=(ci == 0),
                            stop=(ci == DC - 1),
                        )
                    nc.scalar.activation(
                        qT_sb[:, co, :], psq[:], AF.Identity,
                        bias=bqk_sb[:, co : co + 1], scale=1.0,
                    )
                qT_tiles[bc] = qT_sb

            def kloop(bc):
                qT_sb = qT_tiles.pop(bc)
                psu = ps_up.tile([16, 2, 512], f32)  # U accumulator over k
                for h in range(K // KH):
                    r_sb = r_tiles[bc][h]
                    rT_sb = rT_tiles[bc][h]
                    # transpose all candidates of the half on the PE (no xbar:
                    # its transfer time blocks all other DMA traffic)
                    for kl in range(KH):
                        for t in range(2):
                            psr = ps_sm.tile([128, DC, 128], bf16, tag="small")
                            for c in range(DC):
                                nc.tensor.transpose(
                                    psr[:, c], r_sb[:, t, kl, ts(c, 128)], idn_sb[:]
                                )
                            nc.vector.tensor_copy(
                                rT_sb[:, kl, :, ts(t, 128)], psr[:]
                            )
                    for kl in range(KH):
                        k = h * KH + kl
                        pss = ps_sc.tile([128, 2, S], f32)
                        for qc in range(2):
                            for c in range(DC):
                                nc.tensor.matmul(
                                    pss[:, qc],
                                    lhsT=qT_sb[:, c, ts(qc, 128)],
                                    rhs=rT_sb[:, kl, c, :],
                                    start=(c == 0),
                                    stop=(c == DC - 1),
                                )
                        E_sb = pair.tile([128, 2, S], bf16)
                        rs_sb = pair.tile([128, 2], f32)
                        ri_sb = pair.tile([128, 2], f32)
                        w_sb = pair.tile([128, 2, P16], bf16)
                        for qc in range(2):
                            nc.scalar.activation(
                                E_sb[:, qc], pss[:, qc], AF.Exp,
                                accum_out=rs_sb[:, qc : qc + 1],
                            )
                            nc.vector.reciprocal(
                                ri_sb[:, qc : qc + 1], rs_sb[:, qc : qc + 1]
                            )
                            nc.vector.tensor_scalar_mul(
                                w_sb[:, qc], msk_sb[:, qc], ri_sb[:, qc : qc + 1]
                            )
                        # pooled probs (transposed): Pp.T[s,p] = sum_q E[q,s] w[q,p]
                        psp = ps_sm.tile([128, 2, P16], f32, tag="small")
                        for sc in range(2):
                            for qc in range(2):
                                nc.tensor.matmul(
                                    psp[:, sc],
                                    lhsT=E_sb[:, qc, ts(sc, 128)],
                                    rhs=w_sb[:, qc],
                                    start=(qc == 0),
                                    stop=(qc == 1),
                                )
                        ppT_sb = pair.tile([128, 2, P16], bf16)
                        for sc in range(2):
                            nc.vector.tensor_copy(ppT_sb[:, sc], psp[:, sc])
                        # U += Pp @ r   (accumulate over k in PSUM)
                        for sc in range(2):
                            st = k == 0 and sc == 0
                            sp = k == K - 1 and sc == 1
                            nc.tensor.matmul(
                                psu[:, 0, :],
                                lhsT=ppT_sb[:, sc],
                                rhs=r_sb[:, sc, kl, 0:512],
                                start=st, stop=sp, skip_group_check=True,
                            )
                            nc.tensor.matmul(
                                psu[:, 1, 0:256],
                                lhsT=ppT_sb[:, sc],
                                rhs=r_sb[:, sc, kl, 512:768],
                                start=st, stop=sp, skip_group_check=True,
                            )
                del r_tiles[bc], rT_tiles[bc]
                # ---- evict U (transposition deferred to the epilogue) ----
                nc.vector.tensor_copy(U_all[:, bc, 0:512], psu[:, 0, :])
                nc.vector.tensor_copy(U_all[:, bc, 512:768], psu[:, 1, 0:256])

            # software-pipelined schedule: r loads run two bc ahead,
            # prologue one bc ahead
            issue_x_load(0)
            issue_x_load(1)
            issue_r_loads(0)
            prologue(0)
            issue_r_loads(1)
            for bc in range(NBC):
                if bc + 2 < NBC:
                    issue_x_load(bc + 2)
                if bc + 1 < NBC:
                    prologue(bc + 1)
                if bc + 2 < NBC:
                    issue_r_loads(bc + 2)
                kloop(bc)

            # ---- epilogue: build U.T for all (call,batch) ----
            for bc in range(NBC):
                for c in range(DC):
                    pst2 = ps_sm.tile([128, P16], bf16, tag="small")
                    nc.tensor.transpose(
                        pst2[:], U_all[:, bc, ts(c, 128)], idn_sb[0:16, 0:16]
                    )
                    nc.vector.tensor_copy(uT_all[:, c, bc, :], pst2[:])

            # ---- final: out.T = Wv @ U.T + bv, all (call,batch) columns at once ----
            fT_sb = persist.tile([128, DC, NBC * P16], f32)
            for co in range(DC):
                psf = ps_sm.tile([128, NBC * P16], f32, tag="small")
                for ci in range(DC):
                    nc.tensor.matmul(
                        psf[:],
                        lhsT=wvt_sb[:, ci, ts(co, 128)],
                        rhs=uT_all[:, ci],
                        start=(ci == 0),
                        stop=(ci == DC - 1),
                    )
                nc.vector.tensor_scalar_add(fT_sb[:, co], psf[:], bvc_sb[:, co : co + 1])
            nc.sync.dma_start(
                out=outT.rearrange("(c p) n -> p c n", p=128), in_=fT_sb[:]
            )

    nc.compile()
    return nc


def _host_weights(Wq, bq, Wk, Wv, bv):
    scale = 1.0 / np.sqrt(np.float32(D))
    Wqk = (Wq.astype(np.float32).T @ Wk.astype(np.float32)) * scale
    bqk = (bq.astype(np.float32) @ Wk.astype(np.float32)) * scale
    mask = np.zeros((SQ, P16), np.float32)
    mask[np.arange(SQ), np.arange(SQ) // P16] = 1.0 / (P16 * K)
    return {
        "wqk": Wqk.astype(BF16),
        "wvt": np.ascontiguousarray(Wv.astype(np.float32).T).astype(BF16),
        "bqk": np.ascontiguousarray(bqk.reshape(DC, 128).T),
        "bvc": np.ascontiguousarray(bv.astype(np.float32).reshape(DC, 128).T),
        "msk": mask.astype(BF16),
        "idn": np.eye(128, dtype=BF16),
    }


def make_in_maps(V, T, r_i, r_t, Wq, bq, Wk, bk, Wv, bv):
    w = _host_weights(Wq, bq, Wk, Wv, bv)
    in_maps = []
    for c in range(NCORES):
        sl = slice(c * BL, (c + 1) * BL)
        m = dict(w)
        m["xv"] = np.ascontiguousarray(V[sl], dtype=np.float32)
        m["xt"] = np.ascontiguousarray(T[sl], dtype=np.float32)
        m["rv"] = np.ascontiguousarray(r_i[sl], dtype=np.float32)
        m["rt"] = np.ascontiguousarray(r_t[sl], dtype=np.float32)
        in_maps.append(m)
    return in_maps


def assemble(outTs):
    """outTs: list of per-core outT [D, NBC*P16] f32 -> (T_to_T, V_to_V)."""
    Ts, Vs = [], []
    for a in outTs:
        a = a.reshape(D, 2, BL, P16)
        Vs.append(np.ascontiguousarray(a[:, 0].transpose(1, 2, 0)))
        Ts.append(np.ascontiguousarray(a[:, 1].transpose(1, 2, 0)))
    return (
        np.concatenate(Ts, axis=0).astype(np.float32),
        np.concatenate(Vs, axis=0).astype(np.float32),
    )


def get_program():
    if "nc" not in _cache:
        _cache["nc"] = _build_program()
    return _cache["nc"]


def kernel(V, T, r_i, r_t, Wq, bq, Wk, bk, Wv, bv):
    from concourse import bass_utils

    nc = get_program()
    in_maps = make_in_maps(V, T, r_i, r_t, Wq, bq, Wk, bk, Wv, bv)
    res = bass_utils.run_bass_kernel_spmd(nc, in_maps, core_ids=list(range(NCORES)))
    return assemble([r["outT"] for r in res.results])



# revision 11
# speedup vs baseline: 1.3270x; 1.3270x over previous
"""Fused sparse-attention kernel for Trainium2 — 8-core SPMD, data-parallel over batch.

Reference computation (per call, two calls: (V, r_i) and (T, r_t)):
    q      = x @ Wq.T + bq                      # [b,256,768]
    k      = r @ Wk.T + bk                      # [b,8,256,768]
    v      = r @ Wv.T + bv
    S      = (q @ k.T) / sqrt(768)              # [b,8,256,256]
    P      = softmax(S, -1)
    out    = mean_k( pool16(P @ v) )            # pool16: avg over groups of 16 q rows

Algebraic restructuring used here:
  1. softmax is shift-invariant => the bk bias term (constant along s) drops.
     S = q' @ r.T with q' = x @ Wqk + bqk,  Wqk = Wq.T @ Wk * scale (host-precomputed).
     This removes the [b,8,256,768]x[768,768] k-projection entirely.
  2. pool16 is linear => apply it to P before the value matmul:
     pool16(P @ v) = (pool16 P) @ v.  16x fewer FLOPs in the AV matmul.
  3. Wv/bv projection commutes with the k-mean and the pool:
     out = ( mean_k (pool16 P) @ r ) @ Wv.T + bv.  One value projection per batch
     instead of per (batch, k); bv survives unscaled because pooled probs rows sum to 1.
  4. pooling + softmax normalization + k-mean are folded into one tiny matmul:
     Ppooled.T[s, p] = sum_q E[q, s] * M[q, p] / (16*8*rowsum[q]), with
     E = exp(S) and M the 0/1 pool-bin scatter matrix (host constant).
  5. U is accumulated TRANSPOSED: U.T[d, p] += r_tile[s, d].T @ Pp.T[s, p].
     The stationary operand is the natural [s, d] layout of r, so the value
     matmul streams 16-wide outputs instead of 768-wide ones and the final
     projection consumes U.T directly (no eviction transposes).

Per-core dataflow (bf16 matmul inputs, fp32 accumulation):
  - x, r loaded with SWDGE cast f32->bf16 (Pool-engine queue).
  - r.T tiles for the scores matmul are produced per-candidate with PE
    transposes; evictions alternate between Vector and Pool engines.
  - exp on ScalarE reads scores straight from PSUM and emits the softmax
    denominator via accum_out in the same pass.
  - the pooled-prob + value matmuls for candidate k are deferred until after
    the scores matmul of candidate k+1, hiding the softmax cross-engine
    latency behind PE streaming work.
"""

import numpy as np
import ml_dtypes

B, K, S, SQ, D = 32, 8, 256, 256, 768
NCORES = 8
BL = B // NCORES          # batches per core
P16 = 16                  # pooled length
NBC = 2 * BL              # (call, batch) units per core
DC = D // 128             # 6 chunks of the feature dim
KH = 4                    # candidates per r-load tile
BF16 = ml_dtypes.bfloat16

_cache = {}


def _build_program():
    import concourse.bass as bass
    import concourse.bacc as bacc
    import concourse.tile as tile
    import concourse.mybir as mybir

    f32 = mybir.dt.float32
    bf16 = mybir.dt.bfloat16
    ts = bass.ts
    AF = mybir.ActivationFunctionType

    nc = bacc.Bacc("TRN2", target_bir_lowering=False, debug=False)

    xv = nc.dram_tensor("xv", [BL, SQ, D], f32, kind="ExternalInput").ap()
    xt = nc.dram_tensor("xt", [BL, SQ, D], f32, kind="ExternalInput").ap()
    rv = nc.dram_tensor("rv", [BL, K, S, D], f32, kind="ExternalInput").ap()
    rt = nc.dram_tensor("rt", [BL, K, S, D], f32, kind="ExternalInput").ap()
    wqk = nc.dram_tensor("wqk", [D, D], bf16, kind="ExternalInput").ap()
    wvt = nc.dram_tensor("wvt", [D, D], bf16, kind="ExternalInput").ap()
    bqk = nc.dram_tensor("bqk", [128, DC], f32, kind="ExternalInput").ap()
    bvc = nc.dram_tensor("bvc", [128, DC], f32, kind="ExternalInput").ap()
    msk = nc.dram_tensor("msk", [SQ, P16], bf16, kind="ExternalInput").ap()
    idn = nc.dram_tensor("idn", [128, 128], bf16, kind="ExternalInput").ap()
    outT = nc.dram_tensor("outT", [D, NBC * P16], f32, kind="ExternalOutput").ap()

    with tile.TileContext(nc) as tc:
        with (
            tc.tile_pool(name="const", bufs=1) as const,
            tc.tile_pool(name="persist", bufs=1) as persist,
            tc.tile_pool(name="bcpool", bufs=2) as bcp,
            tc.tile_pool(name="xpool", bufs=3) as xpool,
            tc.tile_pool(name="rpool", bufs=7) as rpool,
            tc.tile_pool(name="rtpool", bufs=4) as rtpool,
            tc.tile_pool(name="pair", bufs=6) as pair,
            tc.tile_pool(name="ps_sc", bufs=3, space="PSUM") as ps_sc,
            tc.tile_pool(name="ps_tr", bufs=2, space="PSUM") as ps_tr,
            tc.tile_pool(name="ps_ut", bufs=1, space="PSUM") as ps_ut,
            tc.tile_pool(name="ps_sm", bufs=2, space="PSUM") as ps_sm,
        ):
            # ---- constants, ordered by first use (sync queue is serial) ----
            idn_sb = const.tile([128, 128], bf16)
            nc.sync.dma_start(idn_sb[:], idn[:])
            bqk_sb = const.tile([128, DC], f32)
            nc.sync.dma_start(bqk_sb[:], bqk[:])
            msk_sb = const.tile([128, 2, P16], bf16)
            nc.sync.dma_start(msk_sb[:], msk.rearrange("(t p) m -> p t m", p=128))
            wqk_sb = const.tile([128, DC, D], bf16)
            nc.sync.dma_start(wqk_sb[:], wqk.rearrange("(c p) d -> p c d", p=128))
            bvc_sb = const.tile([128, DC], f32)
            nc.sync.dma_start(bvc_sb[:], bvc[:])
            wvt_sb = const.tile([128, DC, D], bf16)
            nc.sync.dma_start(wvt_sb[:], wvt.rearrange("(c p) d -> p c d", p=128))

            # transposed, Wv-unprojected pooled outputs for every (call, batch)
            uT_all = persist.tile([128, DC, NBC, P16], bf16)
            # zeros operand for the psu-clearing matmul: a start=True matmul
            # marks its whole 2KB PSUM bank pending-zero, so the 6 sub-bank
            # U.T regions must be zeroed by ONE matmul, not 6 start flags.
            zer = const.tile([128, 128], bf16)
            nc.gpsimd.memset(zer[:], 0.0)

            def dram_xr(bc):
                call, b = bc // BL, bc % BL
                return (xv, xt)[call][b], (rv, rt)[call][b]

            x_tiles, r_tiles, qT_tiles = {}, {}, {}

            def issue_x_load(bc):
                x_dram, _ = dram_xr(bc)
                x_sb = xpool.tile([128, 2, D], bf16, tag="xsb")
                nc.gpsimd.dma_start(
                    out=x_sb[:], in_=x_dram.rearrange("(t p) d -> p t d", p=128)
                )
                x_tiles[bc] = x_sb

            def issue_r_loads(bc, fine=False):
                _, r_dram = dram_xr(bc)
                r_tiles[bc] = []
                for h in range(K // KH):
                    r_sb = rpool.tile([128, 2, KH, D], bf16, tag="rsb")
                    if fine:
                        # per-candidate transfers: shortens the pipeline
                        # ramp (first bc) and drain (last bc)
                        for kl in range(KH):
                            k = h * KH + kl
                            nc.gpsimd.dma_start(
                                out=r_sb[:, :, kl, :],
                                in_=r_dram[k].rearrange("(t p) d -> p t d", p=128),
                            )
                    else:
                        r_re = r_dram[h * KH : (h + 1) * KH].rearrange(
                            "k (t p) d -> p t k d", p=128
                        )
                        for t in range(2):
                            nc.gpsimd.dma_start(out=r_sb[:, t], in_=r_re[:, t])
                    r_tiles[bc].append(r_sb)

            def prologue(bc):
                # x.T via PE transposes, then q'.T = Wqk.T @ x.T (+bias on eviction)
                x_sb = x_tiles.pop(bc)
                xT_sb = bcp.tile([128, DC, SQ], bf16, tag="xT")
                for t in range(2):
                    pst = ps_tr.tile([128, DC, 128], bf16, tag="tr")
                    for c in range(DC):
                        nc.tensor.transpose(
                            pst[:, c], x_sb[:, t, ts(c, 128)], idn_sb[:]
                        )
                    nc.vector.tensor_copy(xT_sb[:, :, ts(t, 128)], pst[:])
                qT_sb = bcp.tile([128, DC, SQ], bf16, tag="qT")
                for co in range(DC):
                    psq = ps_sm.tile([128, SQ], f32, tag="small")
                    for ci in range(DC):
                        nc.tensor.matmul(
                            psq[:],
                            lhsT=wqk_sb[:, ci, ts(co, 128)],
                            rhs=xT_sb[:, ci, :],
                            start=(ci == 0),
                            stop=(ci == DC - 1),
                        )
                    nc.scalar.activation(
                        qT_sb[:, co, :], psq[:], AF.Identity,
                        bias=bqk_sb[:, co : co + 1], scale=1.0,
                    )
                qT_tiles[bc] = qT_sb

            # deferred pooled-prob + value work for the previous candidate:
            # emitted after the NEXT candidate's scores so the PE never waits
            # on the exp/recip/mask chain. The U.T accumulator is allocated at
            # first use (k==0) — allocating it earlier would let the 1-buf ring
            # slot's WAR dependency miss the deferred k==7 reads of the
            # previous bc (tile deps only cover reads emitted before reuse).
            pend = {}
            psu_ref = {}

            def flush_pend():
                if not pend:
                    return
                E_sb, w_sb, r_sb = pend["E"], pend["w"], pend["r"]
                kl, k, bc = pend["kl"], pend["k"], pend["bc"]
                pend.clear()
                if k == 0:
                    psu_ref[bc] = ps_ut.tile([128, DC, P16], f32, name="psu")
                    nc.tensor.matmul(
                        psu_ref[bc][:, :, :],
                        lhsT=zer[:, :],
                        rhs=zer[:, 0 : DC * P16],
                        start=True, stop=True, skip_group_check=True,
                    )
                psu = psu_ref[bc]
                # pooled probs (transposed): Pp.T[s,p] = sum_q E[q,s] w[q,p]
                psp = ps_sm.tile([128, 2, P16], f32, tag="small")
                for sc in range(2):
                    for qc in range(2):
                        nc.tensor.matmul(
                            psp[:, sc],
                            lhsT=E_sb[:, qc, ts(sc, 128)],
                            rhs=w_sb[:, qc],
                            start=(qc == 0),
                            stop=(qc == 1),
                        )
                ppT = pair.tile([128, 2, P16], bf16, tag="ppT")
                nc.vector.tensor_copy(ppT[:], psp[:])
                # U.T[d,p] += r_tile.T @ Pp.T  (accumulate over k, sc in PSUM;
                # start=False always — the bank was zeroed by the matmul above)
                for sc in range(2):
                    sp = k == K - 1 and sc == 1
                    for c in range(DC):
                        nc.tensor.matmul(
                            psu[:, c],
                            lhsT=r_sb[:, sc, kl, ts(c, 128)],
                            rhs=ppT[:, sc],
                            start=False, stop=sp, skip_group_check=True,
                        )
                if k == K - 1:
                    nc.vector.tensor_copy(uT_all[:, :, bc, :], psu[:])
                    del psu_ref[bc]

            def kloop(bc):
                qT_sb = qT_tiles.pop(bc)
                for h in range(K // KH):
                    r_sb = r_tiles[bc][h]
                    for kl in range(KH):
                        k = h * KH + kl
                        # transpose candidate kl on the PE (per-candidate
                        # granularity interleaves the Vector evictions with the
                        # softmax-chain work instead of bursting them)
                        rT = rtpool.tile([128, DC, S], bf16, tag="rt")
                        for t in range(2):
                            psr = ps_tr.tile([128, DC, 128], bf16, tag="tr")
                            for c in range(DC):
                                nc.tensor.transpose(
                                    psr[:, c], r_sb[:, t, kl, ts(c, 128)], idn_sb[:]
                                )
                            nc.vector.tensor_copy(rT[:, :, ts(t, 128)], psr[:])
                        # scores
                        pss = ps_sc.tile([128, 2, S], f32)
                        for qc in range(2):
                            for c in range(DC):
                                nc.tensor.matmul(
                                    pss[:, qc],
                                    lhsT=qT_sb[:, c, ts(qc, 128)],
                                    rhs=rT[:, c, :],
                                    start=(c == 0),
                                    stop=(c == DC - 1),
                                )
                        # previous candidate's pooled-prob + value matmuls
                        flush_pend()
                        # softmax chain for this candidate (Scalar + Vector)
                        E_sb = pair.tile([128, 2, S], bf16, tag="E")
                        rs_sb = pair.tile([128, 2], f32, tag="rs")
                        ri_sb = pair.tile([128, 2], f32, tag="ri")
                        w_sb = pair.tile([128, 2, P16], bf16, tag="w")
                        for qc in range(2):
                            nc.scalar.activation(
                                E_sb[:, qc], pss[:, qc], AF.Exp,
                                accum_out=rs_sb[:, qc : qc + 1],
                            )
                            nc.vector.reciprocal(
                                ri_sb[:, qc : qc + 1], rs_sb[:, qc : qc + 1]
                            )
                            nc.vector.tensor_scalar_mul(
                                w_sb[:, qc], msk_sb[:, qc], ri_sb[:, qc : qc + 1]
                            )
                        pend.update(E=E_sb, w=w_sb, r=r_sb, kl=kl, k=k, bc=bc)
                del r_tiles[bc]

            # schedule: loads run two bc ahead; x(bc) lands just before r(bc);
            # prologue(bc+1) fills the PE while the k7 softmax chain drains.
            issue_x_load(0)
            issue_r_loads(0, fine=True)
            issue_x_load(1)
            issue_r_loads(1)
            prologue(0)
            for bc in range(NBC):
                if bc + 2 < NBC:
                    issue_x_load(bc + 2)
                    issue_r_loads(bc + 2, fine=(bc + 2 == NBC - 1))
                kloop(bc)
                if bc + 1 < NBC:
                    prologue(bc + 1)
            flush_pend()

            # ---- final: out.T = Wv @ U.T + bv, all (call,batch) columns at once ----
            fT_sb = persist.tile([128, DC, NBC * P16], f32)
            for co in range(DC):
                psf = ps_sm.tile([128, NBC * P16], f32, tag="small")
                for ci in range(DC):
                    nc.tensor.matmul(
                        psf[:],
                        lhsT=wvt_sb[:, ci, ts(co, 128)],
                        rhs=uT_all[:, ci],
                        start=(ci == 0),
                        stop=(ci == DC - 1),
                    )
                nc.vector.tensor_scalar_add(fT_sb[:, co], psf[:], bvc_sb[:, co : co + 1])
            nc.sync.dma_start(
                out=outT.rearrange("(c p) n -> p c n", p=128), in_=fT_sb[:]
            )

    nc.compile()
    return nc


def _host_weights(Wq, bq, Wk, Wv, bv):
    scale = 1.0 / np.sqrt(np.float32(D))
    Wqk = (Wq.astype(np.float32).T @ Wk.astype(np.float32)) * scale
    bqk = (bq.astype(np.float32) @ Wk.astype(np.float32)) * scale
    mask = np.zeros((SQ, P16), np.float32)
    mask[np.arange(SQ), np.arange(SQ) // P16] = 1.0 / (P16 * K)
    return {
        "wqk": Wqk.astype(BF16),
        "wvt": np.ascontiguousarray(Wv.astype(np.float32).T).astype(BF16),
        "bqk": np.ascontiguousarray(bqk.reshape(DC, 128).T),
        "bvc": np.ascontiguousarray(bv.astype(np.float32).reshape(DC, 128).T),
        "msk": mask.astype(BF16),
        "idn": np.eye(128, dtype=BF16),
    }


def make_in_maps(V, T, r_i, r_t, Wq, bq, Wk, bk, Wv, bv):
    w = _host_weights(Wq, bq, Wk, Wv, bv)
    in_maps = []
    for c in range(NCORES):
        sl = slice(c * BL, (c + 1) * BL)
        m = dict(w)
        m["xv"] = np.ascontiguousarray(V[sl], dtype=np.float32)
        m["xt"] = np.ascontiguousarray(T[sl], dtype=np.float32)
        m["rv"] = np.ascontiguousarray(r_i[sl], dtype=np.float32)
        m["rt"] = np.ascontiguousarray(r_t[sl], dtype=np.float32)
        in_maps.append(m)
    return in_maps


def assemble(outTs):
    """outTs: list of per-core outT [D, NBC*P16] f32 -> (T_to_T, V_to_V)."""
    Ts, Vs = [], []
    for a in outTs:
        a = a.reshape(D, 2, BL, P16)
        Vs.append(np.ascontiguousarray(a[:, 0].transpose(1, 2, 0)))
        Ts.append(np.ascontiguousarray(a[:, 1].transpose(1, 2, 0)))
    return (
        np.concatenate(Ts, axis=0).astype(np.float32),
        np.concatenate(Vs, axis=0).astype(np.float32),
    )


def get_program():
    if "nc" not in _cache:
        _cache["nc"] = _build_program()
    return _cache["nc"]


def kernel(V, T, r_i, r_t, Wq, bq, Wk, bk, Wv, bv):
    from concourse import bass_utils

    nc = get_program()
    in_maps = make_in_maps(V, T, r_i, r_t, Wq, bq, Wk, bk, Wv, bv)
    res = bass_utils.run_bass_kernel_spmd(nc, in_maps, core_ids=list(range(NCORES)))
    return assemble([r["outT"] for r in res.results])


# revision 22
# speedup vs baseline: 1.3678x; 1.0308x over previous
"""Fused sparse-attention kernel for Trainium2 — 8-core SPMD, data-parallel over batch.

Reference computation (per call, two calls: (V, r_i) and (T, r_t)):
    q      = x @ Wq.T + bq                      # [b,256,768]
    k      = r @ Wk.T + bk                      # [b,8,256,768]
    v      = r @ Wv.T + bv
    S      = (q @ k.T) / sqrt(768)              # [b,8,256,256]
    P      = softmax(S, -1)
    out    = mean_k( pool16(P @ v) )            # pool16: avg over groups of 16 q rows

Algebraic restructuring used here:
  1. softmax is shift-invariant => the bk bias term (constant along s) drops.
     S = q' @ r.T with q' = x @ Wqk + bqk,  Wqk = Wq.T @ Wk * scale (host-precomputed).
     This removes the [b,8,256,768]x[768,768] k-projection entirely.
  2. pool16 is linear => apply it to P before the value matmul:
     pool16(P @ v) = (pool16 P) @ v.  16x fewer FLOPs in the AV matmul.
  3. Wv/bv projection commutes with the k-mean and the pool:
     out = ( mean_k (pool16 P) @ r ) @ Wv.T + bv.  One value projection per batch
     instead of per (batch, k); bv survives unscaled because pooled probs rows sum to 1.
  4. pooling + softmax normalization + k-mean are folded into one tiny matmul:
     Ppooled.T[s, p] = sum_q E[q, s] * M[q, p] / (16*8*rowsum[q]), with
     E = exp(S) and M the 0/1 pool-bin scatter matrix (host constant).
  5. U is accumulated TRANSPOSED: U.T[d, p] += r_tile[s, d].T @ Pp.T[s, p].
     The stationary operand is the natural [s, d] layout of r, so the value
     matmul streams 16-wide outputs instead of 768-wide ones and the final
     projection consumes U.T directly (no eviction transposes).

Per-core dataflow (bf16 matmul inputs, fp32 accumulation):
  - x, r loaded with SWDGE cast f32->bf16 (Pool-engine queue).
  - r.T tiles for the scores matmul are produced per-candidate with PE
    transposes; evictions alternate between Vector and Pool engines.
  - exp on ScalarE reads scores straight from PSUM and emits the softmax
    denominator via accum_out in the same pass.
  - the pooled-prob + value matmuls for candidate k are deferred until after
    the scores matmul of candidate k+1, hiding the softmax cross-engine
    latency behind PE streaming work.
"""

import numpy as np
import ml_dtypes

B, K, S, SQ, D = 32, 8, 256, 256, 768
NCORES = 8
BL = B // NCORES          # batches per core
P16 = 16                  # pooled length
NBC = 2 * BL              # (call, batch) units per core
DC = D // 128             # 6 chunks of the feature dim
KH = 4                    # candidates per r-load tile
BF16 = ml_dtypes.bfloat16

_cache = {}


def _build_program():
    import concourse.bass as bass
    import concourse.bacc as bacc
    import concourse.tile as tile
    import concourse.mybir as mybir

    f32 = mybir.dt.float32
    bf16 = mybir.dt.bfloat16
    ts = bass.ts
    AF = mybir.ActivationFunctionType

    nc = bacc.Bacc("TRN2", target_bir_lowering=False, debug=False)

    xv = nc.dram_tensor("xv", [BL, SQ, D], f32, kind="ExternalInput").ap()
    xt = nc.dram_tensor("xt", [BL, SQ, D], f32, kind="ExternalInput").ap()
    rv = nc.dram_tensor("rv", [BL, K, S, D], f32, kind="ExternalInput").ap()
    rt = nc.dram_tensor("rt", [BL, K, S, D], f32, kind="ExternalInput").ap()
    wqk = nc.dram_tensor("wqk", [D, D], bf16, kind="ExternalInput").ap()
    wvt = nc.dram_tensor("wvt", [D, D], bf16, kind="ExternalInput").ap()
    bqk = nc.dram_tensor("bqk", [128, DC], f32, kind="ExternalInput").ap()
    bvc = nc.dram_tensor("bvc", [128, DC], f32, kind="ExternalInput").ap()
    msk = nc.dram_tensor("msk", [SQ, P16], bf16, kind="ExternalInput").ap()
    idn = nc.dram_tensor("idn", [128, 128], bf16, kind="ExternalInput").ap()
    outT = nc.dram_tensor("outT", [D, NBC * P16], f32, kind="ExternalOutput").ap()

    with tile.TileContext(nc) as tc:
        with (
            tc.tile_pool(name="const", bufs=1) as const,
            tc.tile_pool(name="persist", bufs=1) as persist,
            tc.tile_pool(name="bcpool", bufs=2) as bcp,
            tc.tile_pool(name="xpool", bufs=3) as xpool,
            tc.tile_pool(name="rpool", bufs=7) as rpool,
            tc.tile_pool(name="rtpool", bufs=4) as rtpool,
            tc.tile_pool(name="pair", bufs=6) as pair,
            tc.tile_pool(name="ps_sc", bufs=2, space="PSUM") as ps_sc,
            tc.tile_pool(name="ps_tr", bufs=3, space="PSUM") as ps_tr,
            tc.tile_pool(name="ps_ut", bufs=1, space="PSUM") as ps_ut,
            tc.tile_pool(name="ps_sm", bufs=2, space="PSUM") as ps_sm,
        ):
            # ---- constants, ordered by first use (sync queue is serial) ----
            idn_sb = const.tile([128, 128], bf16)
            nc.sync.dma_start(idn_sb[:], idn[:])
            bqk_sb = const.tile([128, DC], f32)
            nc.sync.dma_start(bqk_sb[:], bqk[:])
            msk_sb = const.tile([128, 2, P16], bf16)
            nc.sync.dma_start(msk_sb[:], msk.rearrange("(t p) m -> p t m", p=128))
            wqk_sb = const.tile([128, DC, D], bf16)
            nc.sync.dma_start(wqk_sb[:], wqk.rearrange("(c p) d -> p c d", p=128))
            bvc_sb = const.tile([128, DC], f32)
            nc.sync.dma_start(bvc_sb[:], bvc[:])
            # wvt is only needed by the epilogue — its load is issued mid-kernel
            # (after kloop(1)) so it doesn't compete with the startup x/r DMA
            wvt_sb = const.tile([128, DC, D], bf16)

            # transposed, Wv-unprojected pooled outputs for every (call, batch)
            uT_all = persist.tile([128, DC, NBC, P16], bf16)
            # zeros operand for the psu-clearing matmul: a start=True matmul
            # marks its whole 2KB PSUM bank pending-zero, so the 6 sub-bank
            # U.T regions must be zeroed by ONE matmul, not 6 start flags.
            zer = const.tile([128, 128], bf16)
            nc.gpsimd.memset(zer[:], 0.0)

            def dram_xr(bc):
                call, b = bc // BL, bc % BL
                return (xv, xt)[call][b], (rv, rt)[call][b]

            x_tiles, r_tiles, qT_tiles = {}, {}, {}

            def issue_x_load(bc):
                x_dram, _ = dram_xr(bc)
                x_sb = xpool.tile([128, 2, D], bf16, tag="xsb")
                nc.gpsimd.dma_start(
                    out=x_sb[:], in_=x_dram.rearrange("(t p) d -> p t d", p=128)
                )
                x_tiles[bc] = x_sb

            def issue_r_loads(bc, fine=False):
                _, r_dram = dram_xr(bc)
                r_tiles[bc] = []
                for h in range(K // KH):
                    r_sb = rpool.tile([128, 2, KH, D], bf16, tag="rsb")
                    if fine:
                        # per-candidate transfers: shortens the pipeline
                        # ramp (first bc) and drain (last bc)
                        for kl in range(KH):
                            k = h * KH + kl
                            nc.gpsimd.dma_start(
                                out=r_sb[:, :, kl, :],
                                in_=r_dram[k].rearrange("(t p) d -> p t d", p=128),
                            )
                    else:
                        r_re = r_dram[h * KH : (h + 1) * KH].rearrange(
                            "k (t p) d -> p t k d", p=128
                        )
                        for t in range(2):
                            nc.gpsimd.dma_start(out=r_sb[:, t], in_=r_re[:, t])
                    r_tiles[bc].append(r_sb)

            def prologue(bc):
                # x.T via PE transposes, then q'.T = Wqk.T @ x.T (+bias on eviction)
                x_sb = x_tiles.pop(bc)
                xT_sb = bcp.tile([128, DC, SQ], bf16, tag="xT")
                for t in range(2):
                    pst = ps_tr.tile([128, DC, 128], bf16, tag="tr")
                    for c in range(DC):
                        nc.tensor.transpose(
                            pst[:, c], x_sb[:, t, ts(c, 128)], idn_sb[:]
                        )
                    nc.vector.tensor_copy(xT_sb[:, :, ts(t, 128)], pst[:])
                qT_sb = bcp.tile([128, DC, SQ], bf16, tag="qT")
                for co in range(DC):
                    psq = ps_sm.tile([128, SQ], f32, tag="small")
                    for ci in range(DC):
                        nc.tensor.matmul(
                            psq[:],
                            lhsT=wqk_sb[:, ci, ts(co, 128)],
                            rhs=xT_sb[:, ci, :],
                            start=(ci == 0),
                            stop=(ci == DC - 1),
                        )
                    nc.scalar.activation(
                        qT_sb[:, co, :], psq[:], AF.Identity,
                        bias=bqk_sb[:, co : co + 1], scale=1.0,
                    )
                qT_tiles[bc] = qT_sb

            # deferred pooled-prob + value work for the previous candidate:
            # emitted after the NEXT candidate's scores so the PE never waits
            # on the exp/recip/mask chain. The U.T accumulator is allocated at
            # first use (k==0) — allocating it earlier would let the 1-buf ring
            # slot's WAR dependency miss the deferred k==7 reads of the
            # previous bc (tile deps only cover reads emitted before reuse).
            pend = {}
            psu_ref = {}

            def flush_pend():
                if not pend:
                    return
                E_sb, w_sb, r_sb = pend["E"], pend["w"], pend["r"]
                kl, k, bc = pend["kl"], pend["k"], pend["bc"]
                pend.clear()
                if k == 0:
                    psu_ref[bc] = ps_ut.tile([128, DC, P16], f32, name="psu")
                    nc.tensor.matmul(
                        psu_ref[bc][:, :, :],
                        lhsT=zer[:, :],
                        rhs=zer[:, 0 : DC * P16],
                        start=True, stop=True, skip_group_check=True,
                    )
                psu = psu_ref[bc]
                # pooled probs (transposed): Pp.T[s,p] = sum_q E[q,s] w[q,p]
                psp = ps_sm.tile([128, 2, P16], f32, tag="small")
                for sc in range(2):
                    for qc in range(2):
                        nc.tensor.matmul(
                            psp[:, sc],
                            lhsT=E_sb[:, qc, ts(sc, 128)],
                            rhs=w_sb[:, qc],
                            start=(qc == 0),
                            stop=(qc == 1),
                        )
                ppT = pair.tile([128, 2, P16], bf16, tag="ppT")
                nc.vector.tensor_copy(ppT[:], psp[:])
                # U.T[d,p] += r_tile.T @ Pp.T  (accumulate over k, sc in PSUM;
                # start=False always — the bank was zeroed by the matmul above)
                for sc in range(2):
                    sp = k == K - 1 and sc == 1
                    for c in range(DC):
                        nc.tensor.matmul(
                            psu[:, c],
                            lhsT=r_sb[:, sc, kl, ts(c, 128)],
                            rhs=ppT[:, sc],
                            start=False, stop=sp, skip_group_check=True,
                        )
                if k == K - 1:
                    nc.vector.tensor_copy(uT_all[:, :, bc, :], psu[:])
                    del psu_ref[bc]

            def kloop(bc):
                qT_sb = qT_tiles.pop(bc)
                for h in range(K // KH):
                    r_sb = r_tiles[bc][h]
                    for kl in range(KH):
                        k = h * KH + kl
                        # transpose candidate kl on the PE (per-candidate
                        # granularity interleaves the Vector evictions with the
                        # softmax-chain work instead of bursting them)
                        rT = rtpool.tile([128, DC, S], bf16, tag="rt")
                        for t in range(2):
                            psr = ps_tr.tile([128, DC, 128], bf16, tag="tr")
                            for c in range(DC):
                                nc.tensor.transpose(
                                    psr[:, c], r_sb[:, t, kl, ts(c, 128)], idn_sb[:]
                                )
                            nc.vector.tensor_copy(rT[:, :, ts(t, 128)], psr[:])
                        # scores
                        pss = ps_sc.tile([128, 2, S], f32)
                        for qc in range(2):
                            for c in range(DC):
                                nc.tensor.matmul(
                                    pss[:, qc],
                                    lhsT=qT_sb[:, c, ts(qc, 128)],
                                    rhs=rT[:, c, :],
                                    start=(c == 0),
                                    stop=(c == DC - 1),
                                )
                        # previous candidate's pooled-prob + value matmuls
                        flush_pend()
                        # softmax chain for this candidate (Scalar + Vector)
                        E_sb = pair.tile([128, 2, S], bf16, tag="E")
                        rs_sb = pair.tile([128, 2], f32, tag="rs")
                        ri_sb = pair.tile([128, 2], f32, tag="ri")
                        w_sb = pair.tile([128, 2, P16], bf16, tag="w")
                        for qc in range(2):
                            nc.scalar.activation(
                                E_sb[:, qc], pss[:, qc], AF.Exp,
                                accum_out=rs_sb[:, qc : qc + 1],
                            )
                            nc.vector.reciprocal(
                                ri_sb[:, qc : qc + 1], rs_sb[:, qc : qc + 1]
                            )
                            nc.vector.tensor_scalar_mul(
                                w_sb[:, qc], msk_sb[:, qc], ri_sb[:, qc : qc + 1]
                            )
                        pend.update(E=E_sb, w=w_sb, r=r_sb, kl=kl, k=k, bc=bc)
                del r_tiles[bc]

            # schedule: loads run two bc ahead; x(bc) lands just before r(bc);
            # prologue(bc+1) fills the PE while the k7 softmax chain drains.
            issue_x_load(0)
            issue_r_loads(0, fine=True)
            issue_x_load(1)
            issue_r_loads(1)
            prologue(0)
            for bc in range(NBC):
                if bc + 2 < NBC:
                    issue_x_load(bc + 2)
                    issue_r_loads(bc + 2, fine=(bc + 2 == NBC - 1))
                kloop(bc)
                if bc == 1:
                    nc.sync.dma_start(
                        wvt_sb[:], wvt.rearrange("(c p) d -> p c d", p=128)
                    )
                if bc + 1 < NBC:
                    prologue(bc + 1)
            flush_pend()

            # ---- final: out.T = Wv @ U.T + bv, all (call,batch) columns at once;
            # each feature chunk is stored as soon as its bias lands so the
            # output DMA overlaps the remaining matmuls ----
            outT_re = outT.rearrange("(c p) n -> p c n", p=128)
            fT_sb = persist.tile([128, DC, NBC * P16], f32)
            for co in range(DC):
                psf = ps_sm.tile([128, NBC * P16], f32, tag="small")
                for ci in range(DC):
                    nc.tensor.matmul(
                        psf[:],
                        lhsT=wvt_sb[:, ci, ts(co, 128)],
                        rhs=uT_all[:, ci],
                        start=(ci == 0),
                        stop=(ci == DC - 1),
                    )
                nc.vector.tensor_scalar_add(fT_sb[:, co], psf[:], bvc_sb[:, co : co + 1])
                nc.sync.dma_start(out=outT_re[:, co, :], in_=fT_sb[:, co])

    nc.compile()
    return nc


def _host_weights(Wq, bq, Wk, Wv, bv):
    scale = 1.0 / np.sqrt(np.float32(D))
    Wqk = (Wq.astype(np.float32).T @ Wk.astype(np.float32)) * scale
    bqk = (bq.astype(np.float32) @ Wk.astype(np.float32)) * scale
    mask = np.zeros((SQ, P16), np.float32)
    mask[np.arange(SQ), np.arange(SQ) // P16] = 1.0 / (P16 * K)
    return {
        "wqk": Wqk.astype(BF16),
        "wvt": np.ascontiguousarray(Wv.astype(np.float32).T).astype(BF16),
        "bqk": np.ascontiguousarray(bqk.reshape(DC, 128).T),
        "bvc": np.ascontiguousarray(bv.astype(np.float32).reshape(DC, 128).T),
        "msk": mask.astype(BF16),
        "idn": np.eye(128, dtype=BF16),
    }


def make_in_maps(V, T, r_i, r_t, Wq, bq, Wk, bk, Wv, bv):
    w = _host_weights(Wq, bq, Wk, Wv, bv)
    in_maps = []
    for c in range(NCORES):
        sl = slice(c * BL, (c + 1) * BL)
        m = dict(w)
        m["xv"] = np.ascontiguousarray(V[sl], dtype=np.float32)
        m["xt"] = np.ascontiguousarray(T[sl], dtype=np.float32)
        m["rv"] = np.ascontiguousarray(r_i[sl], dtype=np.float32)
        m["rt"] = np.ascontiguousarray(r_t[sl], dtype=np.float32)
        in_maps.append(m)
    return in_maps


def assemble(outTs):
    """outTs: list of per-core outT [D, NBC*P16] f32 -> (T_to_T, V_to_V)."""
    Ts, Vs = [], []
    for a in outTs:
        a = a.reshape(D, 2, BL, P16)
        Vs.append(np.ascontiguousarray(a[:, 0].transpose(1, 2, 0)))
        Ts.append(np.ascontiguousarray(a[:, 1].transpose(1, 2, 0)))
    return (
        np.concatenate(Ts, axis=0).astype(np.float32),
        np.concatenate(Vs, axis=0).astype(np.float32),
    )


def get_program():
    if "nc" not in _cache:
        _cache["nc"] = _build_program()
    return _cache["nc"]


def kernel(V, T, r_i, r_t, Wq, bq, Wk, bk, Wv, bv):
    from concourse import bass_utils

    nc = get_program()
    in_maps = make_in_maps(V, T, r_i, r_t, Wq, bq, Wk, bk, Wv, bv)
    res = bass_utils.run_bass_kernel_spmd(nc, in_maps, core_ids=list(range(NCORES)))
    return assemble([r["outT"] for r in res.results])
